# revision 1
# baseline (speedup 1.0000x reference)
"""Incremental MADE autoregressive sampler on 8 TRN2 NeuronCores.

Key idea: with hidden units degree-sorted, activations are APPEND-ONLY across
autoregressive steps (unit of degree d never changes once x_0..x_d are set).
So each step only computes the ~33 NEW hidden units per layer (one degree
group), not the whole prefix:

- L1 pre-activations for the currently-growing 128-block live in PSUM and
  receive one rank-1 (K=1) update per step with the new coordinate.
- L2/L3 recompute only the touched 128-block(s): contract the valid a1/a2
  prefix (masked weights make over-inclusion safe: garbage units hit zero
  weights).
- The output layer is a PERSISTENT PSUM accumulator theta[64, B]: when a
  degree group of a3 becomes final it is contracted once (weights zero-masked
  to that group) into ALL future output rows. Step idx just reads rows idx
  (mu) and 32+idx (log_std) directly from PSUM.
- Tail: es=exp(ls) (ACT) -> t2=es*z (DVE/Pool) -> xi=t2+mu (DVE/Pool) ->
  one-hot scatter matmul into xT (off critical path).

Batch is data-parallel over 8 cores (512 rows/core), and each core splits its
batch into two column "chains" of 256 so two independent dependency chains
overlap on the engines (f32r needs N>=256 for 1 cycle/row).
"""

import os
import sys
import math
import numpy as np

for _p in ("/opt/trn_rl_repo", "/opt/pypackages"):
    if _p not in sys.path:
        sys.path.insert(0, _p)

import concourse.bass as bass
import concourse.tile as tile
from concourse import bacc
from concourse import mybir
from concourse.bass_utils import run_bass_kernel_spmd

D, H, L, B = 32, 1024, 2, 4096
NCORES = 8
BC = B // NCORES          # 512 batch rows per core
P = 128                   # partitions
NB = H // P               # 8 hidden blocks
CW = BC // 2              # chain width (256)
F32 = mybir.dt.float32
MMDT = mybir.dt.bfloat16

STOP = int(os.environ.get("MADE_STOP", "32"))
NCH = int(os.environ.get("MADE_CHAINS", "2"))
SKEW = int(os.environ.get("MADE_SKEW", "6"))


def _schedule():
    """Static per-step schedule from the degree structure."""
    d_hid = np.arange(H) % (D - 1)
    perm = np.argsort(d_hid, kind="stable")
    ds = d_hid[perm]
    grp_lo = [int(np.sum(ds < g)) for g in range(D - 1)]
    grp_hi = [int(np.sum(ds <= g)) for g in range(D - 1)]
    covers = {}
    for idx in range(1, D):
        lo, hi = grp_lo[idx - 1], grp_hi[idx - 1]
        covers[idx] = list(range(lo // P, (hi - 1) // P + 1))
    first = {}
    for idx in range(1, D):
        for Bb in covers[idx]:
            first.setdefault(Bb, idx)
    return perm, ds, grp_lo, grp_hi, covers, first


def _host_prep(W0, b0, Wh, bh, Wout, bout):
    perm, ds, grp_lo, grp_hi, covers, first = _schedule()
    d_in = np.arange(D)
    d_out = np.arange(D) - 1
    m0 = (ds[:, None] >= d_in[None, :]).astype(np.float32)        # sorted [H, D]
    mh = (ds[:, None] >= ds[None, :]).astype(np.float32)          # sorted [H, H]
    mo = (d_out[:, None] >= ds[None, :]).astype(np.float32)       # [D, H] sorted cols
    mo = np.concatenate([mo, mo], axis=0)                         # [2D, H]

    W0p = m0 * W0[perm, :]                    # [H, D]
    Wh0p = mh * Wh[0][perm][:, perm]          # [H, H] (out, in)
    Wh1p = mh * Wh[1][perm][:, perm]
    Wop = mo * Wout[:, perm]                  # [2D, H]
    b0p = b0[perm]
    bh0p = bh[0][perm]
    bh1p = bh[1][perm]

    w0T = np.ascontiguousarray(W0p.T)                             # [32, H]
    wh0T = Wh0p.T.reshape(NB, P, H).copy()                        # [c][128, H]
    wh1T = Wh1p.T.reshape(NB, P, H).copy()
    b0L = b0p.reshape(NB, P).T.copy()                             # [128, 8]
    bh0L = bh0p.reshape(NB, P).T.copy()
    bh1L = bh1p.reshape(NB, P).T.copy()

    # K=1 rank-1 L1 update slices: per (idx, B in cover): W0p[block B, idx-1]
    k1_off = {}
    k1_list = []
    for idx in range(1, D):
        for Bb in covers[idx]:
            k1_off[(idx, Bb)] = len(k1_list)
            k1_list.append(W0p[Bb * P:(Bb + 1) * P, idx - 1])
    w0k1 = np.concatenate(k1_list).reshape(1, -1)                 # [1, n*128]

    # theta accumulation slices: per (idx, B in cover): Wop[:, block B].T with
    # unit-rows outside group idx-1 zeroed -> [128, 64]
    wos_off = {}
    wos_list = []
    for idx in range(1, D):
        lo, hi = grp_lo[idx - 1], grp_hi[idx - 1]
        for Bb in covers[idx]:
            blk = Wop[:, Bb * P:(Bb + 1) * P].T.copy()            # [128, 64]
            upos = np.arange(Bb * P, (Bb + 1) * P)
            blk[(upos < lo) | (upos >= hi), :] = 0.0
            wos_off[(idx, Bb)] = len(wos_list)
            wos_list.append(blk)
    wos = np.concatenate(wos_list, axis=1)                        # [128, n*64]

    # per-(idx, B) group-masked (ls, mu) single columns for the extract fold-in
    wpm = np.zeros((P, len(wos_list) * 2), dtype=np.float32)
    for (idx, Bb), off in wos_off.items():
        wpm[:, 2 * off] = wos[:, off * 2 * D + D + idx]           # ls col
        wpm[:, 2 * off + 1] = wos[:, off * 2 * D + idx]           # mu col

    # one-hot extract lhsT: col idx = e_{D+idx} (ls), col D+idx = e_idx (mu)
    ohx = np.zeros((2 * D, 2 * D), dtype=np.float32)
    for idx in range(D):
        ohx[D + idx, idx] = 1.0
        ohx[idx, D + idx] = 1.0

    # hidden block sparsity: contract chunks c<=B with any nonzero weight
    nzh0 = np.zeros((NB, NB), dtype=bool)
    nzh1 = np.zeros((NB, NB), dtype=bool)
    for r in range(NB):
        for c in range(NB):
            nzh0[r, c] = np.any(Wh0p[r * P:(r + 1) * P, c * P:(c + 1) * P])
            nzh1[r, c] = np.any(Wh1p[r * P:(r + 1) * P, c * P:(c + 1) * P])

    return dict(w0T=w0T, wh0T=wh0T, wh1T=wh1T, b0L=b0L, bh0L=bh0L, bh1L=bh1L,
                w0k1=w0k1, wos=wos, wpm=wpm, ohx=ohx,
                k1_off=k1_off, wos_off=wos_off,
                nzh0=nzh0, nzh1=nzh1, covers=covers, first=first,
                bout=bout.astype(np.float32),
                n_k1=len(k1_list), n_wos=len(wos_list))


def _build(prep):
    nc = bacc.Bacc("TRN2", target_bir_lowering=False, debug=False,
                   num_devices=NCORES)

    def din(name, shape, dt=F32):
        return nc.dram_tensor(name, list(shape), dt, kind="ExternalInput").ap()

    d_w0t = din("w0t", (D, H), MMDT)
    d_w0k1 = din("w0k1", (1, prep["n_k1"] * P), MMDT)
    d_wos = din("wos", (P, prep["n_wos"] * 2 * D), MMDT)
    d_wh0 = din("wh0t", (NB, P, H), MMDT)
    d_wh1 = din("wh1t", (NB, P, H), MMDT)
    d_b0 = din("b0l", (P, NB))
    d_bh0 = din("bh0l", (P, NB))
    d_bh1 = din("bh1l", (P, NB))
    d_z = din("zb", (D, BC))
    d_bo = din("boutr", (1, 2 * D), MMDT)
    d_eye = din("eye", (1, D * D), MMDT)
    d_wpm = din("wpm", (P, prep["n_wos"] * 2), MMDT)
    d_ohx = din("ohx", (2 * D, 2 * D), MMDT)
    d_out = nc.dram_tensor("out", [D, BC], F32, kind="ExternalOutput").ap()

    covers, first = prep["covers"], prep["first"]
    k1_off, wos_off = prep["k1_off"], prep["wos_off"]
    nzh0, nzh1 = prep["nzh0"], prep["nzh1"]
    # blocks whose pre1 re-init is emitted at the end of step (first[B]-2)
    reinit_at = {}
    for Bb, f in first.items():
        if f >= 2:
            reinit_at.setdefault(f - 2, []).append(Bb)

    from contextlib import ExitStack
    with tile.TileContext(nc) as tc, ExitStack() as ctx:
        cp = ctx.enter_context(tc.tile_pool(name="const", bufs=1))
        pp = ctx.enter_context(tc.tile_pool(name="pers", bufs=1, space="PSUM"))
        wk = ctx.enter_context(tc.tile_pool(name="work",
                                            bufs=(2 if NCH == 2 else 4),
                                            space="PSUM"))

        w0t = cp.tile([D, H], MMDT, tag="w0t")
        w0k1 = cp.tile([1, prep["n_k1"] * P], MMDT, tag="w0k1")
        wos = cp.tile([P, prep["n_wos"] * 2 * D], MMDT, tag="wos")
        wh0 = [cp.tile([P, H], MMDT, tag=f"wh0_{c}", name=f"wh0_{c}")
               for c in range(NB)]
        wh1 = [cp.tile([P, H], MMDT, tag=f"wh1_{c}", name=f"wh1_{c}")
               for c in range(NB)]
        b0s = cp.tile([P, NB], F32, tag="b0s")
        bh0s = cp.tile([P, NB], F32, tag="bh0s")
        bh1s = cp.tile([P, NB], F32, tag="bh1s")
        bor = cp.tile([1, 2 * D], MMDT, tag="bor")
        oneh = cp.tile([1, D * D], MMDT, tag="oneh")
        ones = cp.tile([1, BC], MMDT, tag="ones")
        zer = cp.tile([P, BC], F32, tag="zer")
        xT = cp.tile([D, BC], MMDT, tag="xT")
        a1 = [cp.tile([P, BC], MMDT, tag=f"a1_{r}", name=f"a1_{r}") for r in range(NB)]
        a2 = [cp.tile([P, BC], MMDT, tag=f"a2_{r}", name=f"a2_{r}") for r in range(NB)]
        a3 = [cp.tile([P, BC], MMDT, tag=f"a3_{r}", name=f"a3_{r}") for r in range(NB)]
        es = cp.tile([1, BC], F32, tag="es")
        t2 = cp.tile([1, BC], F32, tag="t2")
        xi = [cp.tile([1, BC], MMDT, tag=f"xi{p}", name=f"xi{p}")
              for p in range(2)]
        wpm = cp.tile([P, prep["n_wos"] * 2], MMDT, tag="wpm")
        ohx = cp.tile([2 * D, 2 * D], MMDT, tag="ohx")
        theta_sb = cp.tile([2 * D, BC], MMDT, tag="theta_sb")

        # persistent PSUM: 2 ping-pong L1 pre-act banks + ls/mu extract banks
        pre1 = [pp.tile([P, BC], F32, tag=f"pre1_{s}", name=f"pre1_{s}")
                for s in range(2)]
        exl = pp.tile([1, BC], F32, tag="exl")
        exm = pp.tile([1, BC], F32, tag="exm")

        # input DMAs, ordered by first use
        nc.sync.dma_start(bor[:], d_bo)
        nc.sync.dma_start(ohx[:], d_ohx)
        nc.sync.dma_start(w0t[:], d_w0t)
        nc.sync.dma_start(b0s[:], d_b0)
        nc.sync.dma_start(w0k1[:], d_w0k1)
        nc.sync.dma_start(wpm[:], d_wpm)
        nc.sync.dma_start(oneh[:], d_eye)
        nc.sync.dma_start(bh0s[:], d_bh0)
        nc.sync.dma_start(bh1s[:], d_bh1)
        nc.sync.dma_start(wos[:], d_wos)
        for c in range(NB):
            nc.sync.dma_start(wh0[c][:], d_wh0[c, :, :])
            nc.sync.dma_start(wh1[c][:], d_wh1[c, :, :])

        zrow = {}

        def fetch_z(i):
            if i < STOP and i not in zrow:
                zr_t = cp.tile([1, BC], F32, tag="zrow", bufs=4, name=f"zr{i}")
                zrow[i] = zr_t
                nc.sync.dma_start(zr_t[:], d_z[i:i + 1, :])

        for i in range(3):
            fetch_z(i)

        nc.vector.memset(xT[:], 0.0)
        nc.vector.memset(ones[:], 1.0)
        nc.vector.memset(zer[:], 0.0)

        # theta_sb := bout broadcast: K=1 outer product into a work psum, copy
        tps = wk.tile([2 * D, BC], F32, tag="wk0", name="theta_init")
        nc.tensor.matmul(tps, bor[0:1, :], ones[0:1, :], start=True, stop=True)
        nc.scalar.activation(theta_sb[:, :], tps,
                             mybir.ActivationFunctionType.Copy)
        # pre1 slot 0 (block 0) := 0 via matmul from zeroed xT (opens the bank)
        nc.tensor.matmul(pre1[0], w0t[:, 0:P], xT[:, :], start=True, stop=True)

        CWX = BC // NCH
        chs = [(ch, slice(ch * CWX, (ch + 1) * CWX)) for ch in range(NCH)]

        # relu engine per (chain, layer): True -> ACT, False -> DVE
        RELU_ACT = {(0, 1): True, (0, 2): False, (0, 3): True,
                    (1, 1): False, (1, 2): True, (1, 3): True}

        def relu_op(use_act, out_ap, in_ap, bias_ap, hs):
            if use_act:
                nc.scalar.activation(out_ap, in_ap,
                                     mybir.ActivationFunctionType.Relu,
                                     bias=bias_ap, scale=1.0)
            else:
                nc.vector.scalar_tensor_tensor(
                    out_ap, in_ap, bias_ap, zer[:, hs],
                    mybir.AluOpType.add, mybir.AluOpType.max)

        # Build per-chain phase streams; emit interleaved with chain B skewed
        # so B's PE work fills A's dependency stalls (keeps PE p-state high).
        streams = [[] for _ in range(NCH)]

        def ph(ch, fn):
            streams[ch].append(fn)

        l2l3_state = {}

        def emit_hidden(ch, hs, idx, lyr, part):
            """part 0: non-cover chunks; part 1: cover chunks + relu."""
            cov = covers[idx]
            wh, nzh, bsl, a_in, a_outt = {
                2: (wh0, nzh0, bh0s, a1, a2),
                3: (wh1, nzh1, bh1s, a2, a3)}[lyr]
            if part == 0:
                for Bb in cov:
                    chunks = [c for c in range(Bb + 1) if nzh[Bb, c]]
                    ncv = [c for c in chunks if c not in cov]
                    ps = wk.tile([P, BC], F32, tag=f"wk{ch}",
                                 name=f"l{lyr}_{idx}_{ch}_{Bb}")
                    l2l3_state[(ch, lyr, Bb)] = (ps, bool(ncv))
                    for j, c in enumerate(ncv):
                        nc.tensor.matmul(ps[:, hs],
                                         wh[c][:, Bb * P:(Bb + 1) * P],
                                         a_in[c][:, hs],
                                         start=(j == 0), stop=False)
            else:
                for Bb in cov:
                    chunks = [c for c in range(Bb + 1) if nzh[Bb, c]]
                    ccv = [c for c in chunks if c in cov]
                    ps, started = l2l3_state[(ch, lyr, Bb)]
                    for j, c in enumerate(ccv):
                        nc.tensor.matmul(ps[:, hs],
                                         wh[c][:, Bb * P:(Bb + 1) * P],
                                         a_in[c][:, hs],
                                         start=(j == 0 and not started),
                                         stop=(j == len(ccv) - 1))
                for Bb in cov:
                    ps, _ = l2l3_state[(ch, lyr, Bb)]
                    relu_op(RELU_ACT[(ch % 2, lyr)], a_outt[Bb][:, hs],
                            ps[:, hs], bsl[:, Bb:Bb + 1], hs)

        def emit_extract(ch, hs, idx):
            cov = covers.get(idx, [])
            for dst, col in ((exl, idx), (exm, D + idx)):
                seq = [(ohx[:, col:col + 1], theta_sb[:, hs])]
                for Bb in cov:
                    off = wos_off[(idx, Bb)]
                    c = 2 * off if dst is exl else 2 * off + 1
                    seq.append((wpm[:, c:c + 1], a3[Bb][:, hs]))
                for j, (lh, rh) in enumerate(seq):
                    nc.tensor.matmul(dst[0:1, hs], lh, rh,
                                     start=(j == 0), stop=(j == len(seq) - 1))

        def emit_tail(ch, hs, idx):
            nc.scalar.activation(es[0:1, hs], exl[0:1, hs],
                                 mybir.ActivationFunctionType.Exp)
            nc.vector.tensor_tensor(t2[0:1, hs], es[0:1, hs],
                                    zrow[idx][0:1, hs], mybir.AluOpType.mult)
            nc.vector.tensor_tensor(xi[idx % 2][0:1, hs], t2[0:1, hs],
                                    exm[0:1, hs], mybir.AluOpType.add)

        def emit_touch(ch, hs, idx):
            cov = covers.get(idx, [])
            if not cov:
                return
            tmp = wk.tile([2 * D, BC], F32, tag=f"wk{ch}",
                          name=f"th_{idx}_{ch}")
            for j, Bb in enumerate(cov):
                off = wos_off[(idx, Bb)]
                nc.tensor.matmul(tmp[:, hs],
                                 wos[:, off * 2 * D:(off + 1) * 2 * D],
                                 a3[Bb][:, hs],
                                 start=(j == 0), stop=(j == len(cov) - 1))
            nc.vector.tensor_tensor(theta_sb[:, hs], theta_sb[:, hs],
                                    tmp[:, hs], mybir.AluOpType.add)

        def emit_shared(idx):
            """Scatter x_idx into xT, re-inits, z prefetch (full width)."""
            fetch_z(idx + 3)
            nc.sync.dma_start(xT[idx:idx + 1, :], xi[idx % 2][0:1, :])
            for Bb in reinit_at.get(idx, []):
                nc.tensor.matmul(pre1[Bb % 2], w0t[:, Bb * P:(Bb + 1) * P],
                                 xT[:, :], start=True, stop=True)

        for ch, hs in chs:
            last = ch == NCH - 1
            for idx in range(STOP):
                cov = covers.get(idx, [])

                def mk(fn, *args):
                    return lambda a=args: fn(*a)

                if idx >= 1:
                    def p_k1(ch=ch, hs=hs, idx=idx):
                        for Bb in covers[idx]:
                            off = k1_off[(idx, Bb)]
                            nc.tensor.matmul(pre1[Bb % 2][:, hs],
                                             w0k1[0:1, off * P:(off + 1) * P],
                                             xi[(idx - 1) % 2][0:1, hs],
                                             start=False, stop=True,
                                             skip_group_check=True)

                    def p_relu1(ch=ch, hs=hs, idx=idx):
                        for Bb in covers[idx]:
                            relu_op(RELU_ACT[(ch % 2, 1)], a1[Bb][:, hs],
                                    pre1[Bb % 2][:, hs], b0s[:, Bb:Bb + 1], hs)

                    ph(ch, p_k1)
                    ph(ch, p_relu1)
                    ph(ch, lambda: None)
                    ph(ch, mk(emit_hidden, ch, hs, idx, 2, 1))
                    ph(ch, lambda: None)
                    ph(ch, mk(emit_hidden, ch, hs, idx, 3, 1))
                else:
                    for _ in range(6):
                        ph(ch, lambda: None)
                ph(ch, mk(emit_extract, ch, hs, idx))
                ph(ch, mk(emit_tail, ch, hs, idx))
                if last:
                    ph(ch, mk(emit_shared, idx))
                else:
                    ph(ch, lambda: None)
                ph(ch, mk(emit_touch, ch, hs, idx))
                if 1 <= idx + 1 < STOP:
                    ph(ch, mk(emit_hidden, ch, hs, idx + 1, 2, 0))
                    ph(ch, mk(emit_hidden, ch, hs, idx + 1, 3, 0))
                else:
                    ph(ch, lambda: None)
                    ph(ch, lambda: None)

        # interleaved emission with skew
        pos = [0] * NCH
        total = len(streams[0])
        for i in range(total + SKEW * NCH):
            for ch in range(NCH):
                j = i - ch * SKEW
                if 0 <= j < total:
                    streams[ch][j]()
                    pos[ch] = j

        outf = cp.tile([D, BC], F32, tag="outf")
        nc.scalar.activation(outf[:, :], xT[:, :],
                             mybir.ActivationFunctionType.Copy)
        nc.sync.dma_start(d_out, outf[:, :])

    nc.compile()
    return nc


_CACHE = {}


def _get_program(prep):
    if "nc" not in _CACHE:
        _CACHE["nc"] = _build(prep)
    return _CACHE["nc"]


def _in_maps(inputs, prep):
    import ml_dtypes
    bf16 = ml_dtypes.bfloat16
    z = np.asarray(inputs["z"], dtype=np.float32)
    eye = np.eye(D, dtype=np.float32).reshape(1, D * D)
    maps = []
    for c in range(NCORES):
        zs = z[c * BC:(c + 1) * BC, :]                 # [512, 32]
        zbuf = np.ascontiguousarray(zs.T)              # [32, 512]
        maps.append({
            "w0t": prep["w0T"].astype(bf16),
            "w0k1": prep["w0k1"].astype(bf16),
            "wos": prep["wos"].astype(bf16),
            "wh0t": prep["wh0T"].astype(bf16),
            "wh1t": prep["wh1T"].astype(bf16),
            "b0l": prep["b0L"], "bh0l": prep["bh0L"], "bh1l": prep["bh1L"],
            "zb": zbuf, "boutr": prep["bout"][None, :].astype(bf16),
            "eye": eye.astype(bf16),
            "wpm": prep["wpm"].astype(bf16), "ohx": prep["ohx"].astype(bf16),
        })
    return maps


def _prep_from_inputs(inputs):
    return _host_prep(np.asarray(inputs["W0"], np.float32),
                      np.asarray(inputs["b0"], np.float32),
                      np.asarray(inputs["Wh"], np.float32),
                      np.asarray(inputs["bh"], np.float32),
                      np.asarray(inputs["Wout"], np.float32),
                      np.asarray(inputs["bout"], np.float32))


def _run(inputs, trace=False):
    prep = _prep_from_inputs(inputs)
    nc = _get_program(prep)
    maps = _in_maps(inputs, prep)
    res = run_bass_kernel_spmd(nc, maps, core_ids=list(range(NCORES)),
                               trace=trace)
    out = np.empty((B, D), dtype=np.float32)
    for c in range(NCORES):
        out[c * BC:(c + 1) * BC, :] = res.results[c]["out"].T
    return out, res


def kernel(**inputs):
    out, _ = _run(inputs, trace=False)
    return out

